# revision 1
# baseline (speedup 1.0000x reference)
"""Trainium2 Bass kernel for nn_Classifier_59270548685016.

Computes prediction[b, k] = sum(x[b] * classification_images[k]) — i.e. a
[256, 150528] x [150528, 1000] matmul producing [256, 1000] f32.

Strategy:
- Shard the contraction dim D = 3*224*224 = 150528 across the 8 NeuronCores
  (18816 each). Each core computes a partial [256, 1000] f32 product over its
  D-slice; the host sums the 8 partials. Every input byte is read exactly once
  (minimal HBM traffic for any sharding): ~47 MB/core fp16 => ~131 us DMA
  floor vs ~122 us PE floor per core — near-balanced, compute regime.
- Operands are cast to fp16 on the host (randn data: no overflow, ~2^-11
  relative rounding). The PE runs fp16 at the full 1 cycle/row rate and
  accumulates in fp32 PSUM => measured rel err ~2.4e-4.
- Inputs are pre-swizzled on the host into the exact SBUF image layout
  ([128 partitions, k-tile-major free dim]) so every DMA is a plain
  contiguous-per-partition copy at full bandwidth.
- Per core: 147 contraction k-tiles of 128. x^T (9.6 MB) is fully resident in
  SBUF (loaded in 49 chunks so PE starts early); c^T streams through a
  16-deep pool of 3-k-tile chunks (750 KB SWDGE DMAs). Output split
  2x(m=128) x 2x(n=500) PSUM banks; 588 accumulating matmuls, one drain.
- Must be built with bacc.Bacc + nc.compile(): bacc legalizes instructions
  carrying >1 semaphore wait (walrus rejects those — "Too many sync wait
  commands"), which every slot-reuse DMA and the kernel-tail drain need.
"""

import numpy as np

import concourse.bacc as bacc_mod
import concourse.mybir as mybir
import concourse.tile as tile
from concourse.bass_utils import run_bass_kernel_spmd

B = 256                 # batch (output rows)
K = 1000                # classes (output cols)
D = 3 * 224 * 224       # contraction dim, 150528
NCORES = 8
P = 128                 # partitions / PE contraction tile
DC = D // NCORES        # 18816 per-core contraction slice
KT = DC // P            # 147 contraction k-tiles per core
NSPLIT = 2
NTILE = K // NSPLIT     # 500 columns: fits one PSUM bank (<=512 f32)
MT = B // P             # 2 output-row tiles

_CACHE: dict = {}


def _build(kt=KT, kchunk=3, b=B, k=K, ct_bufs=16, xt_chunk=3):
    """Build the per-core module (same program on all 8 cores)."""
    mt = b // P
    ntile = k // NSPLIT
    nchunk = kt // kchunk
    assert kt % kchunk == 0 and kt % xt_chunk == 0 and b % P == 0

    nc = bacc_mod.Bacc("TRN2", debug=False, num_devices=NCORES)
    xt_in = nc.dram_tensor("xt", [P, kt * b], mybir.dt.float16, kind="ExternalInput").ap()
    ct_in = nc.dram_tensor("ct", [P, kt * k], mybir.dt.float16, kind="ExternalInput").ap()
    out = nc.dram_tensor("out", [b, k], mybir.dt.float32, kind="ExternalOutput").ap()

    with tile.TileContext(nc) as tc:
        with (
            tc.tile_pool(name="xtp", bufs=1) as xt_pool,
            tc.tile_pool(name="ctp", bufs=ct_bufs) as ct_pool,
            tc.tile_pool(name="otp", bufs=1) as out_pool,
            tc.tile_pool(name="psp", bufs=1, space="PSUM") as psum_pool,
        ):
            psums = [
                [
                    psum_pool.tile([P, ntile], mybir.dt.float32, tag=f"ps{m}_{n}", name=f"ps{m}_{n}")
                    for n in range(NSPLIT)
                ]
                for m in range(mt)
            ]
            xts = []
            for j in range(kt // xt_chunk):
                xt_sb = xt_pool.tile([P, xt_chunk * b], mybir.dt.float16, tag=f"xt{j}", name=f"xt{j}")
                nc.sync.dma_start(xt_sb, xt_in[:, j * xt_chunk * b:(j + 1) * xt_chunk * b])
                xts.append(xt_sb)

            for j in range(nchunk):
                ct_sb = ct_pool.tile([P, kchunk * k], mybir.dt.float16, tag="ct", name=f"ct{j}")
                nc.gpsimd.dma_start(ct_sb, ct_in[:, j * kchunk * k:(j + 1) * kchunk * k])
                for kk in range(kchunk):
                    ki = j * kchunk + kk
                    xj, xk = divmod(ki, xt_chunk)
                    for m in range(mt):
                        lhsT = xts[xj][:, xk * b + m * P: xk * b + (m + 1) * P]
                        for n in range(NSPLIT):
                            nc.tensor.matmul(
                                psums[m][n],
                                lhsT,
                                ct_sb[:, kk * k + n * ntile: kk * k + (n + 1) * ntile],
                                start=(ki == 0),
                                stop=(ki == kt - 1),
                            )

            for m in range(mt):
                ot = out_pool.tile([P, k], mybir.dt.float32, tag=f"ot{m}", name=f"ot{m}")
                for n in range(NSPLIT):
                    nc.vector.tensor_copy(ot[:, n * ntile:(n + 1) * ntile], psums[m][n])
                nc.sync.dma_start(out[m * P:(m + 1) * P, :], ot)

    nc.compile()
    return nc


def _get_nc():
    if "nc" not in _CACHE:
        _CACHE["nc"] = _build()
    return _CACHE["nc"]


def _prep_in_maps(x, classification_images):
    """Cast to fp16 and swizzle shards into the SBUF image layout.

    xt[p, ki*B + m] = x[m, i*DC + ki*P + p];  ct[p, ki*K + n] = c[n, i*DC + ki*P + p].
    """
    x_flat = np.asarray(x).reshape(B, D).astype(np.float16)
    c_flat = np.asarray(classification_images).reshape(K, D).astype(np.float16)
    in_maps = []
    for i in range(NCORES):
        sl = slice(i * DC, (i + 1) * DC)
        xt = x_flat[:, sl].T.reshape(KT, P, B).transpose(1, 0, 2).reshape(P, KT * B)
        ct = c_flat[:, sl].T.reshape(KT, P, K).transpose(1, 0, 2).reshape(P, KT * K)
        in_maps.append(
            {"xt": np.ascontiguousarray(xt), "ct": np.ascontiguousarray(ct)}
        )
    return in_maps


def _run(in_maps, **kwargs):
    return run_bass_kernel_spmd(_get_nc(), in_maps, core_ids=list(range(NCORES)), **kwargs)


def kernel(x, classification_images):
    in_maps = _prep_in_maps(x, classification_images)
    res = _run(in_maps)
    out = np.zeros((B, K), np.float32)
    for r in res.results:
        out += r["out"]
    return (out,)



# revision 14
# speedup vs baseline: 1.1102x; 1.1102x over previous
"""Trainium2 Bass kernel for nn_Classifier_59270548685016.

Computes prediction[b, k] = sum(x[b] * classification_images[k]) — i.e. a
[256, 150528] x [150528, 1000] matmul producing [256, 1000] f32.

Strategy:
- Shard the contraction dim D = 3*224*224 = 150528 across the 8 NeuronCores
  (18816 each). Each core computes a partial [256, 1000] product over its
  D-slice; the host sums the 8 partials. Every input byte is read exactly once
  (minimal HBM traffic for any sharding).
- x is cast to fp16 (stationary operand); classification_images is cast to
  fp8e3 (E3M4: 4 mantissa bits) — the moving operand. E3M4 streams at the
  full 1 row/cycle PE rate (same as fp16) but halves the dominant DMA stream,
  so the kernel is PE-bound (~122.5 us serial matmul) instead of DMA-bound
  (~134 us at fp16). Rel err is dominated by the e3m4 quantization of c
  (~1.1e-2 < 2e-2 gate); the fp16 x contributes ~2e-4.
- Per core: 147 contraction k-tiles of 128. xt (9.6 MB fp16) is fully SBUF
  resident and streams via SWDGE (gpsimd); ct (18.8 MB fp8e3) streams
  through a deep pool via HWDGE (sync). Separate DGE paths keep the two
  streams' descriptor generation off each other's critical path; small head
  chunks meet the PE's first k-tile deadlines with no stalls.
- Dummy warmup matmuls on a zeroed tile keep the PE's p-state ramp finished
  (and its queue non-empty) before the real stream starts.
- Tail: the last ct chunk is processed quadrant-major so the four PSUM
  drains stagger; copies alternate DVE/ACT (ACT's function table is
  preloaded during warmup); the four output DMAs are spread over
  gpsimd/scalar/gpsimd/sync so no single DGE serializes the tail.
- Output partials are written as fp16 (magnitudes ~4e3 << 65504, rounding
  ~1e-4 relative) to halve the tail DMA; the host accumulates in fp32.
- Must be built with bacc.Bacc + nc.compile(): bacc legalizes instructions
  carrying >1 semaphore wait (walrus rejects those), which every slot-reuse
  DMA and the kernel-tail drain need.
"""

import ml_dtypes
import numpy as np

import concourse.bacc as bacc_mod
import concourse.mybir as mybir
import concourse.tile as tile
from concourse.bass_utils import run_bass_kernel_spmd

B = 256                 # batch (output rows)
K = 1000                # classes (output cols)
D = 3 * 224 * 224       # contraction dim, 150528
NCORES = 8
P = 128                 # partitions / PE contraction tile
DC = D // NCORES        # 18816 per-core contraction slice
KT = DC // P            # 147 contraction k-tiles per core
NSPLIT = 2
NTILE = K // NSPLIT     # 500 columns: fits one PSUM bank (<=512 f32)
MT = B // P             # 2 output-row tiles

_CACHE: dict = {}


def _chunk_schedule(kt, head, body):
    rest = kt - sum(head)
    chunks = list(head) + [body] * (rest // body)
    if rest % body:
        chunks.append(rest % body)
    assert sum(chunks) == kt
    return chunks


def _build(kt=KT, b=B, k=K, ct_bufs=16, n_warm=400, warm_n=8,
           ct_head=(1, 1, 1, 1, 2), xt_head=(2, 2)):
    """Build the per-core module (same program on all 8 cores)."""
    mt = b // P
    ntile = k // NSPLIT
    assert b % P == 0

    nc = bacc_mod.Bacc("TRN2", debug=False, num_devices=NCORES)
    xt_in = nc.dram_tensor("xt", [P, kt * b], mybir.dt.float16, kind="ExternalInput").ap()
    ct_in = nc.dram_tensor("ct", [P, kt * k], mybir.dt.float8e3, kind="ExternalInput").ap()
    out = nc.dram_tensor("out", [b, k], mybir.dt.float16, kind="ExternalOutput").ap()

    ct_chunks = _chunk_schedule(kt, list(ct_head), 3)
    ct_starts = np.cumsum([0] + ct_chunks).tolist()
    xt_chunks = _chunk_schedule(kt, list(xt_head), 3)
    xt_starts = np.cumsum([0] + xt_chunks).tolist()
    ki2x = []
    for jx, nkt in enumerate(xt_chunks):
        for off in range(nkt):
            ki2x.append((jx, off))

    with tile.TileContext(nc) as tc:
        with (
            tc.tile_pool(name="xtp", bufs=1) as xt_pool,
            tc.tile_pool(name="ctp", bufs=ct_bufs) as ct_pool,
            tc.tile_pool(name="otp", bufs=1) as out_pool,
            tc.tile_pool(name="psp", bufs=1, space="PSUM") as psum_pool,
        ):
            psums = [
                [
                    psum_pool.tile([P, ntile], mybir.dt.float32, tag=f"ps{m}_{n}", name=f"ps{m}_{n}")
                    for n in range(NSPLIT)
                ]
                for m in range(mt)
            ]

            # ct chunks stream on the sync (HWDGE) queue; xt chunks via
            # SWDGE (gpsimd) — independent DGE paths, so the first k-tiles
            # of both streams land in parallel.
            ct_tiles = []
            for j in range(len(ct_chunks)):
                ct_sb = ct_pool.tile(
                    [P, ct_chunks[j] * k], mybir.dt.float8e3, tag="ct", name=f"ct{j}"
                )
                nc.sync.dma_start(ct_sb, ct_in[:, ct_starts[j] * k:ct_starts[j + 1] * k])
                ct_tiles.append(ct_sb)
                if j == 0:
                    # xt k-tiles 0-1 right behind ct chunk 0.
                    xts = []
                    xt0 = xt_pool.tile(
                        [P, xt_chunks[0] * b], mybir.dt.float16, tag="xt0", name="xt0"
                    )
                    nc.gpsimd.dma_start(xt0, xt_in[:, 0:xt_chunks[0] * b])
                    xts.append(xt0)
                    # PE warmup: the cost model's p-state ramp needs ~3 us of
                    # uninterrupted PE activity to reach full rate, and it
                    # restarts whenever the engine queue runs dry. Dummy
                    # matmuls on a zeroed tile keep the PE saturated while
                    # the first input chunks are in flight. A tiny ACT copy
                    # also preloads the activation function table (1283 ns)
                    # used by the tail drain.
                    warm_sb = out_pool.tile([P, P], mybir.dt.float16, tag="warm", name="warm")
                    nc.vector.memset(warm_sb, 0)
                    warm_ot = out_pool.tile([P, 8], mybir.dt.float16, tag="wot", name="wot")
                    nc.scalar.copy(warm_ot, warm_sb[:, :8])
                    warm_ps = psum_pool.tile([P, warm_n], mybir.dt.float32, tag="wps", name="wps")
                    for _ in range(n_warm):
                        nc.tensor.matmul(
                            warm_ps, warm_sb[:, :P], warm_sb[:, :warm_n],
                            start=True, stop=True,
                        )
                    for jx in range(1, len(xt_chunks)):
                        xt_sb = xt_pool.tile(
                            [P, xt_chunks[jx] * b], mybir.dt.float16,
                            tag=f"xt{jx}", name=f"xt{jx}",
                        )
                        nc.gpsimd.dma_start(
                            xt_sb, xt_in[:, xt_starts[jx] * b:xt_starts[jx + 1] * b]
                        )
                        xts.append(xt_sb)

            def issue_mm(ki, m, n, ct_sb, kk):
                jx, off = ki2x[ki]
                lhsT = xts[jx][:, off * b + m * P: off * b + (m + 1) * P]
                nc.tensor.matmul(
                    psums[m][n],
                    lhsT,
                    ct_sb[:, kk * k + n * ntile: kk * k + (n + 1) * ntile],
                    start=(ki == 0),
                    stop=(ki == kt - 1),
                )

            nchunk = len(ct_chunks)
            for j in range(nchunk):
                ct_sb = ct_tiles[j]
                last_chunk = j == nchunk - 1
                if not last_chunk:
                    for kk in range(ct_chunks[j]):
                        ki = ct_starts[j] + kk
                        for m in range(mt):
                            for n in range(NSPLIT):
                                issue_mm(ki, m, n, ct_sb, kk)
                else:
                    # Last chunk: quadrant-major so each PSUM drain staggers.
                    # The final quadrant's copy is split across DVE and ACT in
                    # parallel halves to shorten the critical drain chain.
                    copy_engines = ["v", "a", "v", None]
                    dma_engines = [nc.sync, nc.gpsimd, nc.sync, nc.scalar]
                    for m in range(mt):
                        for n in range(NSPLIT):
                            q = m * NSPLIT + n
                            for kk in range(ct_chunks[j]):
                                issue_mm(ct_starts[j] + kk, m, n, ct_sb, kk)
                            ot = out_pool.tile(
                                [P, ntile], mybir.dt.float16, tag=f"ot{m}_{n}", name=f"ot{m}_{n}"
                            )
                            if copy_engines[q] == "v":
                                nc.vector.tensor_copy(ot, psums[m][n])
                            elif copy_engines[q] == "a":
                                nc.scalar.copy(ot, psums[m][n])
                            else:
                                half = ntile // 2
                                nc.vector.tensor_copy(ot[:, :half], psums[m][n][:, :half])
                                nc.scalar.copy(ot[:, half:], psums[m][n][:, half:])
                            dma_engines[q].dma_start(
                                out[m * P:(m + 1) * P, n * ntile:(n + 1) * ntile], ot
                            )

    nc.compile()
    return nc


def _get_nc():
    if "nc" not in _CACHE:
        _CACHE["nc"] = _build()
    return _CACHE["nc"]


def _prep_in_maps(x, classification_images):
    """Cast and swizzle shards into the SBUF image layout.

    xt[p, ki*B + m] = x[m, i*DC + ki*P + p];  ct[p, ki*K + n] = c[n, i*DC + ki*P + p].
    """
    x_flat = np.asarray(x).reshape(B, D).astype(np.float16)
    c_flat = np.asarray(classification_images).reshape(K, D).astype(ml_dtypes.float8_e3m4)
    in_maps = []
    for i in range(NCORES):
        sl = slice(i * DC, (i + 1) * DC)
        xt = x_flat[:, sl].T.reshape(KT, P, B).transpose(1, 0, 2).reshape(P, KT * B)
        ct = c_flat[:, sl].T.reshape(KT, P, K).transpose(1, 0, 2).reshape(P, KT * K)
        in_maps.append(
            {"xt": np.ascontiguousarray(xt), "ct": np.ascontiguousarray(ct)}
        )
    return in_maps


def _run(in_maps, **kwargs):
    return run_bass_kernel_spmd(_get_nc(), in_maps, core_ids=list(range(NCORES)), **kwargs)


def kernel(x, classification_images):
    in_maps = _prep_in_maps(x, classification_images)
    res = _run(in_maps)
    out = np.zeros((B, K), np.float32)
    for r in res.results:
        out += r["out"].astype(np.float32)
    return (out,)


# revision 25
# speedup vs baseline: 1.1134x; 1.0029x over previous
"""Trainium2 Bass kernel for nn_Classifier_59270548685016.

Computes prediction[b, k] = sum(x[b] * classification_images[k]) — i.e. a
[256, 150528] x [150528, 1000] matmul producing [256, 1000] f32.

Strategy:
- Shard the contraction dim D = 3*224*224 = 150528 across the 8 NeuronCores
  (18816 each). Each core computes a partial [256, 1000] product over its
  D-slice; the host sums the 8 partials. Every input byte is read exactly once
  (minimal HBM traffic for any sharding).
- x is cast to fp16 (stationary operand); classification_images is cast to
  fp8e3 (E3M4: 4 mantissa bits) — the moving operand. E3M4 streams at the
  full 1 row/cycle PE rate (same as fp16) but halves the dominant DMA stream,
  so the kernel is PE-bound (~122.5 us serial matmul) instead of DMA-bound
  (~134 us at fp16). Rel err is dominated by the e3m4 quantization of c
  (~1.1e-2 < 2e-2 gate); the fp16 x contributes ~2e-4.
- Per core: 147 contraction k-tiles of 128. xt (9.6 MB fp16) is fully SBUF
  resident and streams via SWDGE (gpsimd); ct (18.8 MB fp8e3) streams
  through a deep pool via HWDGE (sync). Separate DGE paths keep the two
  streams' descriptor generation off each other's critical path; small head
  chunks meet the PE's first k-tile deadlines with zero stalls (verified in
  the timeline simulator).
- Dummy warmup matmuls on a zeroed tile keep the PE's p-state ramp finished
  (and its queue non-empty) before the real stream starts.
- Tail: the last ct chunk is processed quadrant-major so the four PSUM
  drains stagger; copies alternate DVE/ACT (ACT's function table is
  preloaded during warmup); the four output DMAs are spread over
  sync/gpsimd/scalar/sync queues so no single DGE serializes the tail.
- Output partials are written as fp16 (magnitudes ~4e3 << 65504, rounding
  ~1e-4 relative) to halve the tail DMA; the host accumulates in fp32.
- Must be built with bacc.Bacc + nc.compile(): bacc legalizes instructions
  carrying >1 semaphore wait (walrus rejects those), which every slot-reuse
  DMA and the kernel-tail drain need.
"""

import ml_dtypes
import numpy as np

import concourse.bacc as bacc_mod
import concourse.mybir as mybir
import concourse.tile as tile
from concourse.bass_utils import run_bass_kernel_spmd

B = 256                 # batch (output rows)
K = 1000                # classes (output cols)
D = 3 * 224 * 224       # contraction dim, 150528
NCORES = 8
P = 128                 # partitions / PE contraction tile
DC = D // NCORES        # 18816 per-core contraction slice
KT = DC // P            # 147 contraction k-tiles per core
NSPLIT = 2
NTILE = K // NSPLIT     # 500 columns: fits one PSUM bank (<=512 f32)
MT = B // P             # 2 output-row tiles
XH_KT = 2               # leading x k-tiles shipped as fp8e3 (head latency)

_CACHE: dict = {}


def _chunk_schedule(kt, head, body):
    rest = kt - sum(head)
    chunks = list(head) + [body] * (rest // body)
    if rest % body:
        chunks.append(rest % body)
    assert sum(chunks) == kt
    return chunks


def _build(kt=KT, b=B, k=K, ct_bufs=16, n_warm=400, warm_n=8,
           ct_head=(1, 1, 1, 1, 1, 1), xt_head=(2, 2),
           tail_copy="vava", tail_dma="sgas"):
    """Build the per-core module (same program on all 8 cores)."""
    mt = b // P
    ntile = k // NSPLIT
    assert b % P == 0

    nc = bacc_mod.Bacc("TRN2", debug=False, num_devices=NCORES)
    # k-tiles 0..xt_head[0]-1 of x ship as fp8e3 ("xth") so the PE's first
    # lhsT dependency lands sooner; the accuracy cost is ~sqrt(2/147) of the
    # e3m4 noise on x only — far below the ct quantization floor.
    xh_kt = xt_head[0]
    xth_in = nc.dram_tensor("xth", [P, xh_kt * b], mybir.dt.float8e3, kind="ExternalInput").ap()
    xt_in = nc.dram_tensor(
        "xt", [P, (kt - xh_kt) * b], mybir.dt.float16, kind="ExternalInput"
    ).ap()
    ct_in = nc.dram_tensor("ct", [P, kt * k], mybir.dt.float8e3, kind="ExternalInput").ap()
    out = nc.dram_tensor("out", [b, k], mybir.dt.float16, kind="ExternalOutput").ap()

    ct_chunks = _chunk_schedule(kt, list(ct_head), 3)
    ct_starts = np.cumsum([0] + ct_chunks).tolist()
    xt_chunks = _chunk_schedule(kt, list(xt_head), 3)
    xt_starts = np.cumsum([0] + xt_chunks).tolist()
    ki2x = []
    for jx, nkt in enumerate(xt_chunks):
        for off in range(nkt):
            ki2x.append((jx, off))

    with tile.TileContext(nc) as tc:
        with (
            tc.tile_pool(name="xtp", bufs=1) as xt_pool,
            tc.tile_pool(name="ctp", bufs=ct_bufs) as ct_pool,
            tc.tile_pool(name="otp", bufs=1) as out_pool,
            tc.tile_pool(name="psp", bufs=1, space="PSUM") as psum_pool,
        ):
            psums = [
                [
                    psum_pool.tile([P, ntile], mybir.dt.float32, tag=f"ps{m}_{n}", name=f"ps{m}_{n}")
                    for n in range(NSPLIT)
                ]
                for m in range(mt)
            ]

            # ct chunks stream on the sync (HWDGE) queue; xt chunks via
            # SWDGE (gpsimd) — independent DGE paths, so the first k-tiles
            # of both streams land in parallel.
            ct_tiles = []
            for j in range(len(ct_chunks)):
                ct_sb = ct_pool.tile(
                    [P, ct_chunks[j] * k], mybir.dt.float8e3, tag="ct", name=f"ct{j}"
                )
                nc.sync.dma_start(ct_sb, ct_in[:, ct_starts[j] * k:ct_starts[j + 1] * k])
                ct_tiles.append(ct_sb)
                if j == 0:
                    # xt k-tiles 0..xh_kt-1 (fp8e3) right behind ct chunk 0.
                    xts = []
                    xt0 = xt_pool.tile(
                        [P, xh_kt * b], mybir.dt.float8e3, tag="xt0", name="xt0"
                    )
                    nc.gpsimd.dma_start(xt0, xth_in[:, 0:xh_kt * b])
                    xts.append(xt0)
                    # PE warmup: the cost model's p-state ramp needs ~3 us of
                    # uninterrupted PE activity to reach full rate, and it
                    # restarts whenever the engine queue runs dry. Dummy
                    # matmuls on a zeroed tile keep the PE saturated while
                    # the first input chunks are in flight. A tiny ACT copy
                    # also preloads the activation function table (1283 ns)
                    # used by the tail drain.
                    warm_sb = out_pool.tile([P, P], mybir.dt.float16, tag="warm", name="warm")
                    nc.vector.memset(warm_sb, 0)
                    warm_ot = out_pool.tile([P, 8], mybir.dt.float16, tag="wot", name="wot")
                    nc.scalar.copy(warm_ot, warm_sb[:, :8])
                    warm_ps = psum_pool.tile([P, warm_n], mybir.dt.float32, tag="wps", name="wps")
                    for _ in range(n_warm):
                        nc.tensor.matmul(
                            warm_ps, warm_sb[:, :P], warm_sb[:, :warm_n],
                            start=True, stop=True,
                        )
                    for jx in range(1, len(xt_chunks)):
                        xt_sb = xt_pool.tile(
                            [P, xt_chunks[jx] * b], mybir.dt.float16,
                            tag=f"xt{jx}", name=f"xt{jx}",
                        )
                        nc.gpsimd.dma_start(
                            xt_sb,
                            xt_in[:, (xt_starts[jx] - xh_kt) * b:(xt_starts[jx + 1] - xh_kt) * b],
                        )
                        xts.append(xt_sb)

            def issue_mm(ki, m, n, ct_sb, kk):
                jx, off = ki2x[ki]
                lhsT = xts[jx][:, off * b + m * P: off * b + (m + 1) * P]
                nc.tensor.matmul(
                    psums[m][n],
                    lhsT,
                    ct_sb[:, kk * k + n * ntile: kk * k + (n + 1) * ntile],
                    start=(ki == 0),
                    stop=(ki == kt - 1),
                )

            nchunk = len(ct_chunks)
            for j in range(nchunk):
                ct_sb = ct_tiles[j]
                last_chunk = j == nchunk - 1
                if not last_chunk:
                    for kk in range(ct_chunks[j]):
                        ki = ct_starts[j] + kk
                        for m in range(mt):
                            for n in range(NSPLIT):
                                issue_mm(ki, m, n, ct_sb, kk)
                else:
                    # Last chunk: quadrant-major so each PSUM drain staggers.
                    copy_engines = tail_copy
                    dma_engines = [
                        {"s": nc.sync, "a": nc.scalar, "g": nc.gpsimd}[key]
                        for key in tail_dma
                    ]
                    for m in range(mt):
                        for n in range(NSPLIT):
                            q = m * NSPLIT + n
                            for kk in range(ct_chunks[j]):
                                issue_mm(ct_starts[j] + kk, m, n, ct_sb, kk)
                            ot = out_pool.tile(
                                [P, ntile], mybir.dt.float16, tag=f"ot{m}_{n}", name=f"ot{m}_{n}"
                            )
                            if copy_engines[q] == "v":
                                nc.vector.tensor_copy(ot, psums[m][n])
                            else:
                                nc.scalar.copy(ot, psums[m][n])
                            dma_engines[q].dma_start(
                                out[m * P:(m + 1) * P, n * ntile:(n + 1) * ntile], ot
                            )

    nc.compile()
    return nc


def _get_nc():
    if "nc" not in _CACHE:
        _CACHE["nc"] = _build()
    return _CACHE["nc"]


def _prep_in_maps(x, classification_images):
    """Cast and swizzle shards into the SBUF image layout.

    xt[p, ki*B + m] = x[m, i*DC + ki*P + p];  ct[p, ki*K + n] = c[n, i*DC + ki*P + p].
    """
    x_flat = np.asarray(x).reshape(B, D).astype(np.float16)
    c_flat = np.asarray(classification_images).reshape(K, D).astype(ml_dtypes.float8_e3m4)
    in_maps = []
    for i in range(NCORES):
        sl = slice(i * DC, (i + 1) * DC)
        xt3 = x_flat[:, sl].T.reshape(KT, P, B).transpose(1, 0, 2)
        ct = c_flat[:, sl].T.reshape(KT, P, K).transpose(1, 0, 2).reshape(P, KT * K)
        xth = np.ascontiguousarray(
            xt3[:, :XH_KT, :].reshape(P, XH_KT * B)
        ).astype(ml_dtypes.float8_e3m4)
        xt = xt3[:, XH_KT:, :].reshape(P, (KT - XH_KT) * B)
        in_maps.append(
            {
                "xth": xth,
                "xt": np.ascontiguousarray(xt),
                "ct": np.ascontiguousarray(ct),
            }
        )
    return in_maps


def _run(in_maps, **kwargs):
    return run_bass_kernel_spmd(_get_nc(), in_maps, core_ids=list(range(NCORES)), **kwargs)


def kernel(x, classification_images):
    in_maps = _prep_in_maps(x, classification_images)
    res = _run(in_maps)
    out = np.zeros((B, K), np.float32)
    for r in res.results:
        out += r["out"].astype(np.float32)
    return (out,)
